# revision 1
# baseline (speedup 1.0000x reference)
"""Trainium2 Bass kernel for nn_MetaRouter (dense_transformer).

Contract: kernel(**inputs) takes FULL unsharded inputs (as produced by
reference.setup_inputs()) and returns the FULL [B, D] logits, matching
reference.reference(**inputs).

Strategy:
  - Data-parallel over batch: B=16 split as 2 batches per core x 8 cores.
    All parameters replicated. No collectives.
  - Host side: tokens with attention_mask==0 receive softmax weight exactly 0
    for every query (global + all 16 domains use the same mask), so their
    projected states are never consumed. We therefore compact each batch row
    to its unmasked tokens (padded to a multiple of 128) and only stream
    those through the chip. Pad slots get a -1e9 additive score bias, exactly
    reproducing the reference masked-softmax arithmetic.
  - Device side per core:
      ts tile [128 tok, 4096] --SWDGE cast--> bf16 --xbar DMA transpose-->
      [128 feat, 32, 128 tok] ts^T tiles; TensorE matmul against W_proj
      (bf16, fp32 accum) + b_proj as rank-1 matmul; LayerNorm via
      bn_stats/bn_aggr; x stored bf16 [tok, H] and (PE-transposed) x^T
      [H, tok]. Scores = q^T-matmuls over x^T in [query, tok] layout with
      the pad bias applied as a rank-1 matmul; softmax along free dim;
      attention weights transposed back by xbar DMA; context = attn^T x
      matmuls; LN; fused FFN (gelu, LN, gelu) with all weight matrices
      pre-transposed into natural lhsT/rhs layouts; output head folded with
      the temperature on the host.
"""

import os

TSMODE = os.environ.get("KERNEL_TSMODE", "pe")  # pe | dram

import numpy as np
import ml_dtypes

import concourse.bass as bass
import concourse.bacc as bacc
import concourse.tile as tile
from concourse import mybir
from concourse.masks import make_identity

P = 128
H = 512
TOKD = 4096
KC = TOKD // P  # 32 k-chunks of the projection contraction
NQ = 32         # 17 queries (1 global + 16 domains) padded to 32
D = 16
B = 16
S = 2048
N_CORES = 8
B_LOCAL = B // N_CORES
EPS = 1e-5
F32 = mybir.dt.float32
BF16 = mybir.dt.bfloat16


def build_nc(S_c: int, b_out_s: float, skip=frozenset()):
    """Build the per-core Bass program for padded/compacted seq length S_c."""
    assert S_c % P == 0
    NT = S_c // P          # token tiles per batch row
    TT = B_LOCAL * NT      # token tiles per core

    nc = bacc.Bacc("TRN2", target_bir_lowering=False, num_swdge_queues=2)

    ts = nc.declare_dram_parameter("ts", [B_LOCAL * S_c, TOKD], F32, isOutput=False)
    pb = nc.declare_dram_parameter("pb", [1, B_LOCAL * S_c], BF16, isOutput=False)
    wp = nc.declare_dram_parameter("wp", [TOKD, H], BF16, isOutput=False)
    bp = nc.declare_dram_parameter("bp", [1, H], BF16, isOutput=False)
    qt = nc.declare_dram_parameter("qt", [H, NQ], BF16, isOutput=False)
    tg = nc.declare_dram_parameter("tg", [1, H], F32, isOutput=False)
    tb = nc.declare_dram_parameter("tb", [1, H], F32, isOutput=False)
    cg = nc.declare_dram_parameter("cg", [NQ, H], F32, isOutput=False)
    cb = nc.declare_dram_parameter("cb", [NQ, H], F32, isOutput=False)
    fg = nc.declare_dram_parameter("fg", [1, H], F32, isOutput=False)
    fb = nc.declare_dram_parameter("fb", [1, H], F32, isOutput=False)
    w1 = nc.declare_dram_parameter("w1", [2 * H, H], BF16, isOutput=False)
    bf1 = nc.declare_dram_parameter("bf1", [1, H], BF16, isOutput=False)
    w2 = nc.declare_dram_parameter("w2", [H, H], BF16, isOutput=False)
    bf2 = nc.declare_dram_parameter("bf2", [1, H], BF16, isOutput=False)
    wo = nc.declare_dram_parameter("wo", [1, H], F32, isOutput=False)
    out = nc.declare_dram_parameter("out", [B_LOCAL, D], F32, isOutput=True)
    ts_bf = nc.dram_tensor("ts_bf", [B_LOCAL * S_c, TOKD], BF16)

    with tile.TileContext(nc) as tc:
        _emit(tc, nc, S_c, NT, TT, b_out_s, skip,
              ts=ts, pb=pb, wp=wp, bp=bp, qt=qt, tg=tg, tb=tb, cg=cg, cb=cb,
              fg=fg, fb=fb, w1=w1, bf1=bf1, w2=w2, bf2=bf2, wo=wo, out=out,
              ts_bf=ts_bf)
    nc.compile()
    return nc


def _emit(tc, nc, S_c, NT, TT, b_out_s, skip, *, ts, pb, wp, bp, qt, tg, tb, cg, cb,
          fg, fb, w1, bf1, w2, bf2, wo, out, ts_bf):
    from contextlib import ExitStack
    ctx = ExitStack()
    with ctx:
        const = ctx.enter_context(tc.tile_pool(name="const", bufs=1))
        tsp = ctx.enter_context(tc.tile_pool(name="tsp", bufs=4))
        tstp = ctx.enter_context(tc.tile_pool(name="tstp", bufs=3))
        xp = ctx.enter_context(tc.tile_pool(name="xp", bufs=1))
        lnp = ctx.enter_context(tc.tile_pool(name="lnp", bufs=4))
        p2 = ctx.enter_context(tc.tile_pool(name="p2", bufs=1))
        psx = ctx.enter_context(tc.tile_pool(name="psx", bufs=3, space="PSUM"))
        pst = ctx.enter_context(tc.tile_pool(name="pst", bufs=2, space="PSUM"))
        pss_p = ctx.enter_context(tc.tile_pool(name="pss_p", bufs=2, space="PSUM"))

        # ---- W on the HWDGE rings; first tile loads lead the SWDGE queue ----
        w_sb = const.tile([P, KC, H], BF16)
        _wp = wp.ap().rearrange("(c p) h -> p c h", p=P)
        for _q in range(4):
            _qs = slice(_q * (KC // 4), (_q + 1) * (KC // 4))
            eng = nc.sync if _q % 2 == 0 else nc.scalar
            eng.dma_start(out=w_sb[:, _qs, :], in_=_wp[:, _qs, :])
        prefetched = {}
        for _t in range(min(3, TT)):
            if TSMODE == "dram":
                nc.gpsimd.dma_start(out=ts_bf.ap()[_t * P:(_t + 1) * P, :],
                                    in_=ts.ap()[_t * P:(_t + 1) * P, :])
                prefetched[_t] = True
            else:
                _tn = tsp.tile([P, TOKD], BF16, tag="ts_nat")
                nc.gpsimd.dma_start(out=_tn, in_=ts.ap()[_t * P:(_t + 1) * P, :])
                prefetched[_t] = _tn

        # ---- constants ----

        def bcast(dram, parts, dt=F32):
            t = const.tile([parts, H], dt, tag=f"c_{dram.name}")
            a = dram.ap()
            nc.gpsimd.dma_start(
                out=t, in_=bass.AP(tensor=a.tensor, offset=a.offset,
                                   ap=[[0, parts]] + list(a.ap[1:])))
            return t

        tg_sb = bcast(tg, P)
        tb_sb = bcast(tb, P)
        bp_sb = const.tile([1, H], BF16)
        nc.sync.dma_start(out=bp_sb, in_=bp.ap())

        deferred = {}

        def load_p2_consts():
            qt_sb = const.tile([P, 4, NQ], BF16)
            nc.sync.dma_start(out=qt_sb,
                              in_=qt.ap().rearrange("(c p) q -> p c q", p=P))
            w1_sb = const.tile([P, 8, H], BF16)
            nc.sync.dma_start(out=w1_sb,
                              in_=w1.ap().rearrange("(c p) h -> p c h", p=P))
            w2_sb = const.tile([P, 4, H], BF16)
            nc.sync.dma_start(out=w2_sb,
                              in_=w2.ap().rearrange("(c p) h -> p c h", p=P))
            fg_sb = bcast(fg, NQ)
            fb_sb = bcast(fb, NQ)
            wo_sb = bcast(wo, NQ)
            cg_sb = const.tile([NQ, H], F32)
            nc.sync.dma_start(out=cg_sb, in_=cg.ap())
            cb_sb = const.tile([NQ, H], F32)
            nc.sync.dma_start(out=cb_sb, in_=cb.ap())
            bf1_sb = const.tile([1, H], BF16)
            nc.sync.dma_start(out=bf1_sb, in_=bf1.ap())
            bf2_sb = const.tile([1, H], BF16)
            nc.sync.dma_start(out=bf2_sb, in_=bf2.ap())
            pb_sb = const.tile([1, B_LOCAL * S_c], BF16)
            nc.sync.dma_start(out=pb_sb, in_=pb.ap())
            deferred.update(qt_sb=qt_sb, w1_sb=w1_sb, w2_sb=w2_sb, fg_sb=fg_sb,
                            fb_sb=fb_sb, wo_sb=wo_sb, cg_sb=cg_sb, cb_sb=cb_sb,
                            bf1_sb=bf1_sb, bf2_sb=bf2_sb, pb_sb=pb_sb)

        eps_sb = const.tile([P, 1], F32)
        nc.vector.memset(eps_sb, EPS)
        ones_row = const.tile([1, P], BF16)
        nc.vector.memset(ones_row, 1.0)
        ones_col = const.tile([P, D], BF16)
        nc.vector.memset(ones_col, 1.0)
        id128 = const.tile([P, P], BF16)
        make_identity(nc, id128)
        id32 = const.tile([NQ, NQ], BF16)
        make_identity(nc, id32)

        # x (bf16, [tok, H]) and xT (bf16, [H, tok]) both SBUF-resident
        x_sb = xp.tile([P, TT, H], BF16)
        xT_sb = xp.tile([P, B_LOCAL, 4, S_c], BF16)
        logit_sb = xp.tile([NQ, B_LOCAL], F32)

        # ---------------- phase 1: project + LN + transpose ----------------
        # Software pipeline: PE-transpose tile t's ts chunks, then (while
        # DVE/ACT copy them to SBUF) run tile t-1's projection matmuls.
        tsT_tiles = [None] * TT

        def load_and_transpose(t):
            rows = slice(t * P, (t + 1) * P)
            tsT = tstp.tile([P, KC, P], BF16, tag="tsT")
            h = KC // 2
            if TSMODE == "dram":
                if t not in prefetched:
                    nc.gpsimd.dma_start(out=ts_bf.ap()[rows, :],
                                        in_=ts.ap()[rows, :])
                else:
                    prefetched.pop(t)
                nc.sync.dma_start_transpose(tsT[:, :h, :],
                                            ts_bf.ap()[rows, :h * P])
                nc.scalar.dma_start_transpose(tsT[:, h:, :],
                                              ts_bf.ap()[rows, h * P:])
            else:
                if t in prefetched:
                    ts_nat = prefetched.pop(t)
                else:
                    ts_nat = tsp.tile([P, TOKD], BF16, tag="ts_nat")
                    nc.gpsimd.dma_start(out=ts_nat, in_=ts.ap()[rows, :])
                # PE-transpose path
                for g in range(KC // 4):
                    ptr = pst.tile([P, 4 * P], BF16, tag="ptr")
                    for j in range(4):
                        k = 4 * g + j
                        nc.tensor.transpose(ptr[:, j * P:(j + 1) * P],
                                            ts_nat[:, k * P:(k + 1) * P], id128)
                    if g % 2 == 0:
                        nc.vector.tensor_copy(out=tsT[:, 4 * g:4 * (g + 1), :],
                                              in_=ptr)
                    else:
                        nc.scalar.copy(out=tsT[:, 4 * g:4 * (g + 1), :], in_=ptr)
            tsT_tiles[t] = tsT

        def project(t):
            tsT = tsT_tiles[t]
            px = psx.tile([P, H], F32, tag="px")
            for k in range(KC):
                nc.tensor.matmul(px, lhsT=tsT[:, k, :], rhs=w_sb[:, k, :],
                                 start=(k == 0),
                                 stop=(k == KC - 1 and "bp" in skip))
            if "bp" not in skip:
                nc.tensor.matmul(px, lhsT=ones_row, rhs=bp_sb,
                                 start=False, stop=True)
            tsT_tiles[t] = None
            if t >= 1:
                xT_emit(t - 1)

            stats = lnp.tile([P, 6], F32, tag="stats")
            nc.vector.bn_stats(out=stats, in_=px)
            mv = lnp.tile([P, 2], F32, tag="mv")
            nc.vector.bn_aggr(out=mv, in_=stats)
            rstd = lnp.tile([P, 1], F32, tag="rstd")
            nc.scalar.activation(out=rstd, in_=mv[:, 1:2],
                                 func=mybir.ActivationFunctionType.Sqrt,
                                 bias=eps_sb, scale=1.0)
            nc.vector.reciprocal(out=rstd, in_=rstd)
            if "tln" in skip:
                nc.vector.tensor_scalar(out=x_sb[:, t, :], in0=px,
                                        scalar1=mv[:, 0:1], scalar2=rstd,
                                        op0=mybir.AluOpType.subtract,
                                        op1=mybir.AluOpType.mult)
            else:
                xn = lnp.tile([P, H], F32, tag="xn")
                nc.vector.tensor_scalar(out=xn, in0=px, scalar1=mv[:, 0:1],
                                        scalar2=rstd,
                                        op0=mybir.AluOpType.subtract,
                                        op1=mybir.AluOpType.mult)
                xg = lnp.tile([P, H], F32, tag="xg")
                nc.vector.tensor_mul(out=xg, in0=xn, in1=tg_sb)
                nc.vector.tensor_add(out=x_sb[:, t, :], in0=xg, in1=tb_sb)

        def xT_emit(t):
            # x^T via PE transpose (4x [128,128] -> one PSUM bank -> 1 copy)
            pt = pst.tile([P, 4 * P], BF16, tag="ptr")
            for j in range(4):
                nc.tensor.transpose(pt[:, j * P:(j + 1) * P],
                                    x_sb[:, t, j * P:(j + 1) * P], id128)
            b_i, t_i = divmod(t, NT)
            nc.vector.tensor_copy(
                out=xT_sb[:, b_i, :, t_i * P:(t_i + 1) * P], in_=pt)

        st2 = {}

        def p2_scores(b):
            scores = p2.tile([NQ, S_c], F32, tag="scores")
            n_grp = (S_c + H - 1) // H
            for g in range(n_grp):
                w = min(H, S_c - g * H)
                pss = pss_p.tile([NQ, H], F32, tag="ps_small")
                for hc in range(4):
                    nc.tensor.matmul(
                        pss[:, :w], lhsT=deferred["qt_sb"][:, hc, :],
                        rhs=xT_sb[:, b, hc, g * H: g * H + w],
                        start=(hc == 0), stop=False)
                nc.tensor.matmul(
                    pss[:, :w], lhsT=ones_row[:, :NQ],
                    rhs=deferred["pb_sb"][:, b * S_c + g * H: b * S_c + g * H + w],
                    start=False, stop=True)
                nc.vector.tensor_copy(out=scores[:, g * H: g * H + w],
                                      in_=pss[:, :w])
            st2[b] = {"scores": scores}

        def p2_softmax(b):
            scores = st2[b]["scores"]
            nmax = lnp.tile([NQ, 1], F32, tag="nmax")
            nc.vector.reduce_max(out=nmax, in_=scores,
                                 axis=mybir.AxisListType.X, negate=True)
            pexp = p2.tile([NQ, S_c], F32, tag="pexp")
            rsum = lnp.tile([NQ, 1], F32, tag="rsum")
            nc.scalar.activation(out=pexp, in_=scores,
                                 func=mybir.ActivationFunctionType.Exp,
                                 bias=nmax, scale=1.0, accum_out=rsum)
            rinv = lnp.tile([NQ, 1], F32, tag="rinv")
            nc.vector.reciprocal(out=rinv, in_=rsum)
            attn = p2.tile([NQ, S_c], BF16, tag="attn")
            nc.vector.tensor_scalar_mul(out=attn, in0=pexp, scalar1=rinv)
            attnT = p2.tile([P, NT, NQ], BF16, tag="attnT")
            nc.scalar.dma_start_transpose(attnT, attn)
            st2[b]["attnT"] = attnT

        def p2_ctx(b):
            attnT = st2[b]["attnT"]
            pc = pss_p.tile([NQ, H], F32, tag="ps_small")
            for i in range(NT):
                nc.tensor.matmul(pc, lhsT=attnT[:, i, :],
                                 rhs=x_sb[:, b * NT + i, :],
                                 start=(i == 0), stop=(i == NT - 1))

            # LN (row 0: gln, rows 1..16: cln)
            stats = lnp.tile([NQ, 6], F32, tag="stats2")
            nc.vector.bn_stats(out=stats, in_=pc)
            mv = lnp.tile([NQ, 2], F32, tag="mv2")
            nc.vector.bn_aggr(out=mv, in_=stats)
            rstd = lnp.tile([NQ, 1], F32, tag="rstd2")
            nc.scalar.activation(out=rstd, in_=mv[:, 1:2],
                                 func=mybir.ActivationFunctionType.Sqrt,
                                 bias=eps_sb[:NQ], scale=1.0)
            nc.vector.reciprocal(out=rstd, in_=rstd)
            ctxln = p2.tile([NQ, H], BF16, tag="ctxln")
            if "gcln" in skip:
                nc.vector.tensor_scalar(out=ctxln, in0=pc, scalar1=mv[:, 0:1],
                                        scalar2=rstd,
                                        op0=mybir.AluOpType.subtract,
                                        op1=mybir.AluOpType.mult)
            else:
                cn = p2.tile([NQ, H], F32, tag="cn")
                nc.vector.tensor_scalar(out=cn, in0=pc, scalar1=mv[:, 0:1],
                                        scalar2=rstd,
                                        op0=mybir.AluOpType.subtract,
                                        op1=mybir.AluOpType.mult)
                cgn = p2.tile([NQ, H], F32, tag="cgn")
                nc.vector.tensor_mul(out=cgn, in0=cn, in1=deferred["cg_sb"])
                nc.vector.tensor_add(out=ctxln, in0=cgn, in1=deferred["cb_sb"])

            st2[b]["ctxln"] = ctxln

        def p2_ctxT(b):
            ctxln = st2[b]["ctxln"]
            # ctxln^T [H, NQ] via PE transpose
            pct = pst.tile([P, 4 * NQ], BF16, tag="ps2t", bufs=1)
            for j in range(4):
                nc.tensor.transpose(pct[:, j * NQ:(j + 1) * NQ],
                                    ctxln[:, j * P:(j + 1) * P], id32)
            ctxT = p2.tile([P, 4, NQ], BF16, tag="ctxT")
            nc.vector.tensor_copy(out=ctxT, in_=pct)
            gcol = p2.tile([P, 4, 1], F32, tag="gcol")
            nc.vector.tensor_copy(
                out=gcol, in_=pct.rearrange("p (c q) -> p c q", q=NQ)[:, :, 0:1])

            # fused^T [128, 8, 16]: chunks 0-3 = d_ctx^T, 4-7 = g_ctx^T bcast
            fusedT = p2.tile([P, 8, D], BF16, tag="fusedT")
            for c in range(4):
                nc.vector.tensor_copy(out=fusedT[:, c, :], in_=ctxT[:, c, 1:1 + D])
            for c in range(4):
                nc.vector.tensor_scalar_mul(out=fusedT[:, 4 + c, :], in0=ones_col,
                                            scalar1=gcol[:, c, :])
            st2[b]["fusedT"] = fusedT

        def p2_ffn(b):
            fusedT = st2[b]["fusedT"]
            # FFN layer 1: h1 = gelu(fused @ W_ff1 + b_ff1)  [D, H]
            ph1 = pss_p.tile([NQ, H], F32, tag="ps_small")
            for kc in range(8):
                nc.tensor.matmul(ph1[:D, :], lhsT=fusedT[:, kc, :],
                                 rhs=deferred["w1_sb"][:, kc, :], start=(kc == 0),
                                 stop=(kc == 7 and "bf1" in skip))
            if "bf1" not in skip:
                nc.tensor.matmul(ph1[:D, :], lhsT=ones_row[:, :D], rhs=deferred["bf1_sb"],
                                 start=False, stop=True)
            h1 = p2.tile([NQ, H], F32, tag="h1")
            nc.scalar.activation(out=h1[:D, :], in_=ph1[:D, :],
                                 func=mybir.ActivationFunctionType.Gelu)

            # fln LN
            stats = lnp.tile([NQ, 6], F32, tag="stats2")
            nc.vector.bn_stats(out=stats[:D, :], in_=h1[:D, :])
            mv = lnp.tile([NQ, 2], F32, tag="mv2")
            nc.vector.bn_aggr(out=mv[:D, :], in_=stats[:D, :])
            rstd = lnp.tile([NQ, 1], F32, tag="rstd2")
            nc.scalar.activation(out=rstd[:D, :], in_=mv[:D, 1:2],
                                 func=mybir.ActivationFunctionType.Sqrt,
                                 bias=eps_sb[:D], scale=1.0)
            nc.vector.reciprocal(out=rstd[:D, :], in_=rstd[:D, :])
            h1ln = p2.tile([NQ, H], BF16, tag="h1ln")
            if "fln" in skip:
                nc.vector.tensor_scalar(out=h1ln[:D, :], in0=h1[:D, :],
                                        scalar1=mv[:D, 0:1], scalar2=rstd[:D, :],
                                        op0=mybir.AluOpType.subtract,
                                        op1=mybir.AluOpType.mult)
            else:
                h1n = p2.tile([NQ, H], F32, tag="h1n")
                nc.vector.tensor_scalar(out=h1n[:D, :], in0=h1[:D, :],
                                        scalar1=mv[:D, 0:1], scalar2=rstd[:D, :],
                                        op0=mybir.AluOpType.subtract,
                                        op1=mybir.AluOpType.mult)
                h1g = p2.tile([NQ, H], F32, tag="h1g")
                nc.vector.tensor_mul(out=h1g[:D, :], in0=h1n[:D, :],
                                     in1=deferred["fg_sb"][:D, :])
                nc.vector.tensor_add(out=h1ln[:D, :], in0=h1g[:D, :],
                                     in1=deferred["fb_sb"][:D, :])

            st2[b]["h1ln"] = h1ln

        def p2_ffn2(b):
            h1ln = st2[b]["h1ln"]
            # h1ln^T via PE transpose (rows D..NQ of h1ln are stale; isolated)
            ph1t = pst.tile([P, 4 * D], BF16, tag="ps2t", bufs=1)
            for j in range(4):
                nc.tensor.transpose(ph1t[:, j * D:(j + 1) * D],
                                    h1ln[:D, j * P:(j + 1) * P], id32[:D, :D])
            h1T = p2.tile([P, 4, D], BF16, tag="h1T")
            nc.vector.tensor_copy(out=h1T, in_=ph1t)

            # FFN layer 2: h2 = gelu(h1ln @ W_ff2 + b_ff2)  [D, H]
            ph2 = pss_p.tile([NQ, H], F32, tag="ps_small")
            for kc in range(4):
                nc.tensor.matmul(ph2[:D, :], lhsT=h1T[:, kc, :],
                                 rhs=deferred["w2_sb"][:, kc, :], start=(kc == 0),
                                 stop=(kc == 3 and "bf2" in skip))
            if "bf2" not in skip:
                nc.tensor.matmul(ph2[:D, :], lhsT=ones_row[:, :D], rhs=deferred["bf2_sb"],
                                 start=False, stop=True)
            h2 = p2.tile([NQ, H], F32, tag="h2")
            nc.scalar.activation(out=h2[:D, :], in_=ph2[:D, :],
                                 func=mybir.ActivationFunctionType.Gelu)

            # logits = h2 . wo + b_out_s   (wo has 1/temperature folded in)
            prod = p2.tile([NQ, H], F32, tag="prod")
            nc.vector.tensor_mul(out=prod[:D, :], in0=h2[:D, :], in1=deferred["wo_sb"][:D, :])
            lsum = lnp.tile([NQ, 1], F32, tag="lsum")
            nc.vector.reduce_sum(out=lsum[:D, :], in_=prod[:D, :],
                                 axis=mybir.AxisListType.X)
            nc.vector.tensor_scalar_add(out=logit_sb[:D, b:b + 1],
                                        in0=lsum[:D, :], scalar1=float(b_out_s))

        # driver: pipeline tiles; interleave batch b's phase 2 into the
        # following batch's projection stream so PE never waits on softmax.
        stage = [0] * B_LOCAL

        p2_stages = [lambda b: (p2_scores(b), p2_softmax(b)),
                     p2_ctx, p2_ctxT, p2_ffn, p2_ffn2]

        def advance(b):
            p2_stages[stage[b]](b)
            stage[b] += 1

        for t in range(TT):
            load_and_transpose(t)
            if t > 0:
                project(t - 1)
            if t == min(3, TT - 1):
                load_p2_consts()
            done = t - 1  # tiles with x AND x^T fully emitted
            for b in range(B_LOCAL - 1):
                if done >= (b + 1) * NT + stage[b] and stage[b] < len(p2_stages):
                    advance(b)
        project(TT - 1)
        xT_emit(TT - 1)
        for b in range(B_LOCAL):
            while stage[b] < len(p2_stages):
                advance(b)

        nc.sync.dma_start(out=out.ap().rearrange("b d -> d b"), in_=logit_sb[:D, :])


def _np(x):
    return np.asarray(x)


LAST_RESULT = None


def kernel(**inputs):
    from concourse.bass_utils import run_bass_kernel_spmd

    token_states = _np(inputs["token_states"]).astype(np.float32)
    mask = _np(inputs["attention_mask"])
    W_proj = _np(inputs["W_proj"]).astype(np.float32)
    b_proj = _np(inputs["b_proj"]).astype(np.float32)
    tln_g = _np(inputs["tln_g"]).astype(np.float32)
    tln_b = _np(inputs["tln_b"]).astype(np.float32)
    gln_g = _np(inputs["gln_g"]).astype(np.float32)
    gln_b = _np(inputs["gln_b"]).astype(np.float32)
    cln_g = _np(inputs["cln_g"]).astype(np.float32)
    cln_b = _np(inputs["cln_b"]).astype(np.float32)
    fln_g = _np(inputs["fln_g"]).astype(np.float32)
    fln_b = _np(inputs["fln_b"]).astype(np.float32)
    domain_queries = _np(inputs["domain_queries"]).astype(np.float32)
    global_query = _np(inputs["global_query"]).astype(np.float32)
    W_ff1 = _np(inputs["W_ff1"]).astype(np.float32)
    b_ff1 = _np(inputs["b_ff1"]).astype(np.float32)
    W_ff2 = _np(inputs["W_ff2"]).astype(np.float32)
    b_ff2 = _np(inputs["b_ff2"]).astype(np.float32)
    W_out = _np(inputs["W_out"]).astype(np.float32)
    b_out = _np(inputs["b_out"]).astype(np.float32)
    log_temperature = _np(inputs["log_temperature"]).astype(np.float32)

    Bq, Sq = mask.shape
    assert (Bq, Sq) == (B, S) and token_states.shape == (B, S, TOKD)

    # ---- host preprocessing ----
    compact = os.environ.get("KERNEL_COMPACT", "1") == "1"
    if compact:
        counts = mask.astype(bool).sum(axis=1)
        S_c = int(max(128, -(-int(counts.max()) // P) * P))
    else:
        S_c = S

    ts_c = np.zeros((B, S_c, TOKD), np.float32)
    padbias = np.full((B, S_c), -1e9, np.float32)
    if compact:
        for b in range(B):
            idx = np.flatnonzero(mask[b])
            n = len(idx)
            ts_c[b, :n] = token_states[b, idx]
            padbias[b, :n] = 0.0
    else:
        ts_c[:] = token_states
        padbias[:] = np.where(mask != 0, 0.0, -1e9)

    temp = float(np.clip(np.exp(log_temperature[0]), 0.3, 3.0))
    inv_t = 1.0 / temp
    wo_host = (W_out[:, 0] * inv_t).astype(np.float32)
    b_out_s = float(b_out[0] * inv_t)

    q_all = np.concatenate([global_query[None, :], domain_queries], axis=0)  # [17,H]
    qt_host = np.zeros((H, NQ), np.float32)
    qt_host[:, :17] = q_all.T

    cg_host = np.ones((NQ, H), np.float32)
    cb_host = np.zeros((NQ, H), np.float32)
    cg_host[0] = gln_g
    cb_host[0] = gln_b
    cg_host[1:17] = cln_g
    cb_host[1:17] = cln_b

    bf16 = ml_dtypes.bfloat16

    skip = set()
    if np.all(tln_g == 1) and np.all(tln_b == 0):
        skip.add("tln")
    if np.all(cg_host == 1) and np.all(cb_host == 0):
        skip.add("gcln")
    if np.all(fln_g == 1) and np.all(fln_b == 0):
        skip.add("fln")
    if np.all(b_proj == 0):
        skip.add("bp")
    if np.all(b_ff1 == 0):
        skip.add("bf1")
    if np.all(b_ff2 == 0):
        skip.add("bf2")

    nc = build_nc(S_c, b_out_s, frozenset(skip))

    shared = dict(
        pb=None,  # per-core below
        wp=W_proj.astype(bf16),
        bp=b_proj[None, :].astype(bf16),
        qt=qt_host.astype(bf16),
        tg=tln_g[None, :], tb=tln_b[None, :],
        cg=cg_host, cb=cb_host,
        fg=fln_g[None, :], fb=fln_b[None, :],
        w1=W_ff1.astype(bf16), bf1=b_ff1[None, :].astype(bf16),
        w2=W_ff2.astype(bf16), bf2=b_ff2[None, :].astype(bf16),
        wo=wo_host[None, :],
    )

    in_maps = []
    for c in range(N_CORES):
        m = dict(shared)
        bs = slice(c * B_LOCAL, (c + 1) * B_LOCAL)
        m["ts"] = ts_c[bs].reshape(B_LOCAL * S_c, TOKD)
        m["pb"] = padbias[bs].reshape(1, B_LOCAL * S_c).astype(bf16)
        in_maps.append(m)

    trace = os.environ.get("KERNEL_TRACE", "0") == "1"
    kw = {}
    if trace:
        kw = dict(trace=True, tmpdir=os.environ.get("KERNEL_TRACE_DIR") or None)
    res = run_bass_kernel_spmd(nc, in_maps, core_ids=list(range(N_CORES)), **kw)
    global LAST_RESULT
    LAST_RESULT = res
    outs = [res.results[c]["out"] for c in range(N_CORES)]
    return np.concatenate(outs, axis=0).astype(np.float32)


if __name__ == "__main__":
    pass



# revision 6
# speedup vs baseline: 1.3306x; 1.3306x over previous
"""Trainium2 Bass kernel for nn_MetaRouter (dense_transformer).

Contract: kernel(**inputs) takes FULL unsharded inputs (as produced by
reference.setup_inputs()) and returns the FULL [B, D] logits, matching
reference.reference(**inputs).

Strategy:
  - Data-parallel over batch: B=16 split as 2 rows per core x 8 cores.
    All parameters replicated. No collectives.
  - Host side: tokens with attention_mask==0 get softmax weight exactly 0
    for every query, so each row is compacted to its unmasked tokens
    (padded to a multiple of 128; pad slots get a -1e9 score bias).
    ts is pre-cast to bf16 and pre-transposed into [tile, 128 feat, tok]
    chunks so the chip never transposes it. The 17 attention queries are
    folded into the projection weight matrix as extra columns:
        Q_hat = W @ q - w_bar * colsum(q)   (w_bar = row-mean of W)
    which makes raw_score[s,q] = ts_s . Q_hat[:,q] = v_s.q - mu_s*sum(q),
    i.e. the LN mean-correction is pre-applied; only the per-token rstd
    scaling remains. So scores cost 17 extra matmul columns, not a
    separate pass, and x^T never needs to exist on chip.
  - Softmax denominators and the LN mean shift both cancel inside the
    downstream LayerNorms (LN is invariant to positive scaling and
    uniform shifts), so the context sums use unnormalized exp weights
    against the raw (pre-LN) projections, with the per-token rstd folded
    into the exp weights. No reduce_max, no reciprocal, no renorm.
  - Per tile of 128 tokens: 64 matmuls (32 k-chunks x 2 PSUM splits of
    265+264 columns; one PSUM bank each, LDWEIGHTS fully hidden), then
    DVE does bn_stats/bn_aggr + a bitcast-Newton rsqrt (keeps the ACT
    table pinned on Exp), ACT does exp(rstd*raw + padbias) in a single
    fused instruction, and the per-row context accumulates incrementally
    in PSUM via one [128,17]x[128,512] matmul interleaved into the
    projection stream.
  - Tail per row: LN (DVE rsqrt), PE transposes for the FFN operands,
    gelu-FFN with all weights pre-chunked, output head folded with the
    temperature on the host.
"""

import os

import numpy as np
import ml_dtypes

import concourse.bass as bass
import concourse.bacc as bacc
import concourse.tile as tile
from concourse import mybir
from concourse.masks import make_identity

P = 128
H = 512
TOKD = 4096
KC = TOKD // P    # 32 k-chunks of the projection contraction
NQ = 17           # 1 global + 16 domain queries
WTOT = H + NQ     # 529 projection output columns
SPL = 265         # psum split: [0:265) and [265:529)
D = 16
B = 16
S = 2048
N_CORES = 8
B_LOCAL = B // N_CORES
EPS = 1e-5
F32 = mybir.dt.float32
I32 = mybir.dt.int32
BF16 = mybir.dt.bfloat16
MAGIC = 0x5F3759DF


def build_nc(S_c: int, b_out_s: float, skip=frozenset()):
    """Build the per-core Bass program for padded/compacted seq length S_c."""
    assert S_c % P == 0
    NT = S_c // P          # token tiles per batch row
    TT = B_LOCAL * NT      # token tiles per core

    nc = bacc.Bacc("TRN2", target_bir_lowering=False, num_swdge_queues=2)

    ts = nc.declare_dram_parameter("ts", [TT * P, TOKD], BF16, isOutput=False)
    pb = nc.declare_dram_parameter("pb", [TT, P], F32, isOutput=False)
    wp = nc.declare_dram_parameter("wp", [P, KC * WTOT], BF16, isOutput=False)
    bprow = nc.declare_dram_parameter("bprow", [1, WTOT], BF16, isOutput=False)
    tg = nc.declare_dram_parameter("tg", [1, H], F32, isOutput=False)
    tb = nc.declare_dram_parameter("tb", [1, H], F32, isOutput=False)
    cg = nc.declare_dram_parameter("cg", [NQ, H], F32, isOutput=False)
    cb = nc.declare_dram_parameter("cb", [NQ, H], F32, isOutput=False)
    fg = nc.declare_dram_parameter("fg", [1, H], F32, isOutput=False)
    fb = nc.declare_dram_parameter("fb", [1, H], F32, isOutput=False)
    w1 = nc.declare_dram_parameter("w1", [P, 8 * H], BF16, isOutput=False)
    bf1 = nc.declare_dram_parameter("bf1", [1, H], BF16, isOutput=False)
    w2 = nc.declare_dram_parameter("w2", [P, 4 * H], BF16, isOutput=False)
    bf2 = nc.declare_dram_parameter("bf2", [1, H], BF16, isOutput=False)
    wo = nc.declare_dram_parameter("wo", [1, H], F32, isOutput=False)
    out = nc.declare_dram_parameter("out", [B_LOCAL, D], F32, isOutput=True)

    with tile.TileContext(nc) as tc:
        _emit(tc, nc, NT, TT, b_out_s, skip,
              ts=ts, pb=pb, wp=wp, bprow=bprow, tg=tg, tb=tb, cg=cg, cb=cb,
              fg=fg, fb=fb, w1=w1, bf1=bf1, w2=w2, bf2=bf2, wo=wo, out=out)
    nc.compile()
    return nc


def _emit(tc, nc, NT, TT, b_out_s, skip, *, ts, pb, wp, bprow, tg, tb, cg, cb,
          fg, fb, w1, bf1, w2, bf2, wo, out):
    from contextlib import ExitStack
    ctx = ExitStack()
    with ctx:
        const = ctx.enter_context(tc.tile_pool(name="const", bufs=1))
        tsp = ctx.enter_context(tc.tile_pool(name="tsp", bufs=5))
        xp = ctx.enter_context(tc.tile_pool(name="xp", bufs=1))
        lnp = ctx.enter_context(tc.tile_pool(name="lnp", bufs=2))
        p2 = ctx.enter_context(tc.tile_pool(name="p2", bufs=2))
        psx = ctx.enter_context(tc.tile_pool(name="psx", bufs=2, space="PSUM"))
        ctxp = ctx.enter_context(tc.tile_pool(name="ctxp", bufs=2, space="PSUM"))
        pst = ctx.enter_context(tc.tile_pool(name="pst", bufs=1, space="PSUM"))
        ffnp = ctx.enter_context(tc.tile_pool(name="ffnp", bufs=1, space="PSUM"))

        # ---- weights / first ts tiles lead the DMA rings ----
        w_sb = const.tile([P, KC, WTOT], BF16)
        _wp = wp.ap().rearrange("p (c w) -> p c w", w=WTOT)
        nc.sync.dma_start(out=w_sb[:, 0:1, :], in_=_wp[:, 0:1, :])
        nc.scalar.dma_start(out=w_sb[:, 1:4, :], in_=_wp[:, 1:4, :])
        nc.sync.dma_start(out=w_sb[:, 4:16, :], in_=_wp[:, 4:16, :])

        prefetched = {}

        def dma_tile(t, eng):
            tt = tsp.tile([P, KC * P], BF16, tag="ts")
            eng.dma_start(out=tt, in_=ts.ap()[t * P:(t + 1) * P, :])
            prefetched[t] = tt

        dma_tile(0, nc.gpsimd)
        dma_tile(1, nc.gpsimd)
        nc.scalar.dma_start(out=w_sb[:, 16:32, :], in_=_wp[:, 16:32, :])
        dma_tile(2, nc.gpsimd)
        dma_tile(3, nc.gpsimd)

        # ---- constants ----
        def bcast(dram, parts, dt=F32):
            t = const.tile([parts, H], dt, tag=f"c_{dram.name}")
            a = dram.ap()
            nc.sync.dma_start(
                out=t, in_=bass.AP(tensor=a.tensor, offset=a.offset,
                                   ap=[[0, parts]] + list(a.ap[1:])))
            return t

        pb_sb = const.tile([P, TT], F32)
        nc.sync.dma_start(out=pb_sb, in_=pb.ap().rearrange("t p -> p t"))
        w1_sb = const.tile([P, 8, H], BF16)
        nc.sync.dma_start(out=w1_sb, in_=w1.ap().rearrange("p (c h) -> p c h", h=H))
        w2_sb = const.tile([P, 4, H], BF16)
        nc.sync.dma_start(out=w2_sb, in_=w2.ap().rearrange("p (c h) -> p c h", h=H))
        wo_sb = bcast(wo, D)
        if "tln" not in skip:
            tg_sb = bcast(tg, P)
            tb_sb = bcast(tb, P)
        if "gcln" not in skip:
            cg_sb = const.tile([NQ, H], F32)
            nc.sync.dma_start(out=cg_sb, in_=cg.ap())
            cb_sb = const.tile([NQ, H], F32)
            nc.sync.dma_start(out=cb_sb, in_=cb.ap())
        if "fln" not in skip:
            fg_sb = bcast(fg, D)
            fb_sb = bcast(fb, D)
        if "bf1" not in skip:
            bf1_sb = const.tile([1, H], BF16)
            nc.sync.dma_start(out=bf1_sb, in_=bf1.ap())
        if "bf2" not in skip:
            bf2_sb = const.tile([1, H], BF16)
            nc.sync.dma_start(out=bf2_sb, in_=bf2.ap())
        if "bp" not in skip:
            bprow_sb = const.tile([1, WTOT], BF16)
            nc.sync.dma_start(out=bprow_sb, in_=bprow.ap())

        ones_row = const.tile([1, P], BF16)
        nc.vector.memset(ones_row, 1.0)
        ones_col = const.tile([P, D], BF16)
        nc.vector.memset(ones_col, 1.0)
        id17 = const.tile([NQ, NQ], BF16)
        make_identity(nc, id17)
        id16 = const.tile([D, D], BF16)
        make_identity(nc, id16)
        magic = const.tile([P, 1], I32)
        nc.vector.memset(magic, MAGIC)

        # x (raw projection, bf16) + unnormalized-attn weights, SBUF-resident
        x_sb = xp.tile([P, TT, H], BF16)
        pexpT = xp.tile([P, TT, NQ], BF16)
        logit_sb = xp.tile([D, B_LOCAL], F32)

        def rsqrt(ve, parts, tag):
            """y ~= (ve)^-0.5 via bitcast seed + 2 Newton steps (DVE only)."""
            y = lnp.tile([parts, 1], F32, tag=f"y_{tag}")
            sh = lnp.tile([parts, 1], I32, tag=f"sh_{tag}")
            nc.vector.tensor_scalar(out=sh, in0=ve.bitcast(I32), scalar1=1,
                                    scalar2=None,
                                    op0=mybir.AluOpType.arith_shift_right)
            nc.vector.tensor_tensor(out=y.bitcast(I32), in0=magic[:parts],
                                    in1=sh, op=mybir.AluOpType.subtract)
            t1 = lnp.tile([parts, 1], F32, tag=f"t1_{tag}")
            hh = lnp.tile([parts, 1], F32, tag=f"h_{tag}")
            for _ in range(2):
                nc.vector.tensor_mul(out=t1, in0=y, in1=y)
                nc.vector.tensor_mul(out=t1, in0=t1, in1=ve)
                nc.vector.tensor_scalar(out=hh, in0=t1, scalar1=-0.5,
                                        scalar2=1.5, op0=mybir.AluOpType.mult,
                                        op1=mybir.AluOpType.add)
                nc.vector.tensor_mul(out=y, in0=y, in1=hh)
            return y

        psums = {}

        def proj(t):
            tsT = prefetched.pop(t).rearrange("p (c s) -> p c s", s=P)
            pxa = psx.tile([P, SPL], F32, tag="pxa")
            pxb = psx.tile([P, WTOT - SPL], F32, tag="pxb")
            last = "bp" in skip
            for k in range(KC):
                nc.tensor.matmul(pxa, lhsT=tsT[:, k, :], rhs=w_sb[:, k, 0:SPL],
                                 start=(k == 0), stop=(k == KC - 1 and last))
                nc.tensor.matmul(pxb, lhsT=tsT[:, k, :], rhs=w_sb[:, k, SPL:],
                                 start=(k == 0), stop=(k == KC - 1 and last))
            if not last:
                nc.tensor.matmul(pxa, lhsT=ones_row, rhs=bprow_sb[:, 0:SPL],
                                 start=False, stop=True)
                nc.tensor.matmul(pxb, lhsT=ones_row, rhs=bprow_sb[:, SPL:],
                                 start=False, stop=True)
            psums[t] = (pxa, pxb)

        def post(t):
            """Stats + x store + exp-weights for tile t (DVE/ACT work)."""
            pxa, pxb = psums.pop(t)
            stats = lnp.tile([P, 12], F32, tag="stats")
            nc.vector.bn_stats(out=stats[:, 0:6], in_=pxa)
            nc.vector.bn_stats(out=stats[:, 6:12], in_=pxb[:, 0:H - SPL])
            mv = lnp.tile([P, 2], F32, tag="mv")
            nc.vector.bn_aggr(out=mv, in_=stats)
            ve = lnp.tile([P, 1], F32, tag="ve")
            nc.vector.tensor_scalar_add(out=ve, in0=mv[:, 1:2], scalar1=EPS)
            rstd = rsqrt(ve, P, "p1")
            if "tln" in skip:
                # store raw v; rstd folds into the attn weights, mu cancels
                nc.vector.tensor_copy(out=x_sb[:, t, 0:SPL], in_=pxa)
                nc.vector.tensor_copy(out=x_sb[:, t, SPL:H], in_=pxb[:, 0:H - SPL])
            else:
                xa = lnp.tile([P, H], F32, tag="xa")
                nc.vector.tensor_scalar(out=xa[:, 0:SPL], in0=pxa,
                                        scalar1=mv[:, 0:1], scalar2=rstd,
                                        op0=mybir.AluOpType.subtract,
                                        op1=mybir.AluOpType.mult)
                nc.vector.tensor_scalar(out=xa[:, SPL:H], in0=pxb[:, 0:H - SPL],
                                        scalar1=mv[:, 0:1], scalar2=rstd,
                                        op0=mybir.AluOpType.subtract,
                                        op1=mybir.AluOpType.mult)
                xg = lnp.tile([P, H], F32, tag="xg")
                nc.vector.tensor_mul(out=xg, in0=xa, in1=tg_sb)
                nc.vector.tensor_add(out=x_sb[:, t, :], in0=xg, in1=tb_sb)
            # attn weights: exp(rstd*raw_score + padbias) [, * rstd]
            nc.scalar.activation(out=pexpT[:, t, :], in_=pxb[:, H - SPL:],
                                 func=mybir.ActivationFunctionType.Exp,
                                 bias=pb_sb[:, t:t + 1], scale=rstd)
            if "tln" in skip:
                nc.vector.tensor_scalar_mul(out=pexpT[:, t, :],
                                            in0=pexpT[:, t, :], scalar1=rstd)

        ctx_ps = {}

        def ctx_mm(t):
            b, i = divmod(t, NT)
            if i == 0:
                ctx_ps[b] = ctxp.tile([NQ, H], F32, tag="ctx", name="ctx")
            nc.tensor.matmul(ctx_ps[b], lhsT=pexpT[:, t, :], rhs=x_sb[:, t, :],
                             start=(i == 0), stop=(i == NT - 1))

        def row_ctx(b):
            """Context LN + transposes + fused operand build for row b."""
            cps = ctx_ps.pop(b)
            stats = p2.tile([NQ, 6], F32, tag="stats2")
            nc.vector.bn_stats(out=stats, in_=cps)
            mv = p2.tile([NQ, 2], F32, tag="mv2")
            nc.vector.bn_aggr(out=mv, in_=stats)
            ve = p2.tile([NQ, 1], F32, tag="ve2")
            nc.vector.tensor_scalar_add(out=ve, in0=mv[:, 1:2], scalar1=EPS)
            rstd = rsqrt(ve, NQ, "p2")
            ctxln = p2.tile([NQ, H], BF16, tag="ctxln")
            if "gcln" in skip:
                nc.vector.tensor_scalar(out=ctxln, in0=cps, scalar1=mv[:, 0:1],
                                        scalar2=rstd,
                                        op0=mybir.AluOpType.subtract,
                                        op1=mybir.AluOpType.mult)
            else:
                cn = p2.tile([NQ, H], F32, tag="cn")
                nc.vector.tensor_scalar(out=cn, in0=cps, scalar1=mv[:, 0:1],
                                        scalar2=rstd,
                                        op0=mybir.AluOpType.subtract,
                                        op1=mybir.AluOpType.mult)
                cgn = p2.tile([NQ, H], F32, tag="cgn")
                nc.vector.tensor_mul(out=cgn, in0=cn, in1=cg_sb)
                nc.vector.tensor_add(out=ctxln, in0=cgn, in1=cb_sb)

            pt = pst.tile([P, 4, NQ + 1], BF16, tag="tr")
            for j in range(4):
                nc.tensor.transpose(pt[:, j, 0:NQ], ctxln[:, j * P:(j + 1) * P],
                                    id17)
            ctxT = p2.tile([P, 4, NQ], BF16, tag="ctxT")
            nc.vector.tensor_copy(out=ctxT, in_=pt[:, :, 0:NQ])
            gcol = p2.tile([P, 4, 1], F32, tag="gcol")
            nc.vector.tensor_copy(out=gcol, in_=pt[:, :, 0:1])

            fusedT = p2.tile([P, 8, D], BF16, tag="fusedT")
            for c in range(4):
                nc.vector.tensor_copy(out=fusedT[:, c, :], in_=ctxT[:, c, 1:1 + D])
            for c in range(4):
                nc.vector.tensor_scalar_mul(out=fusedT[:, 4 + c, :], in0=ones_col,
                                            scalar1=gcol[:, c, :])
            return fusedT

        def row_ffn(b, fusedT):
            ph1 = ffnp.tile([D, H], F32, tag="ph")
            for kc in range(8):
                nc.tensor.matmul(ph1, lhsT=fusedT[:, kc, :],
                                 rhs=w1_sb[:, kc, :], start=(kc == 0),
                                 stop=(kc == 7 and "bf1" in skip))
            if "bf1" not in skip:
                nc.tensor.matmul(ph1, lhsT=ones_row[:, :D], rhs=bf1_sb,
                                 start=False, stop=True)
            h1 = p2.tile([D, H], F32, tag="h1")
            nc.scalar.activation(out=h1, in_=ph1,
                                 func=mybir.ActivationFunctionType.Gelu)

            stats = p2.tile([D, 6], F32, tag="stats3")
            nc.vector.bn_stats(out=stats, in_=h1)
            mv = p2.tile([D, 2], F32, tag="mv3")
            nc.vector.bn_aggr(out=mv, in_=stats)
            ve = p2.tile([D, 1], F32, tag="ve3")
            nc.vector.tensor_scalar_add(out=ve, in0=mv[:, 1:2], scalar1=EPS)
            rstd = rsqrt(ve, D, "p3")
            h1ln = p2.tile([D, H], BF16, tag="h1ln")
            if "fln" in skip:
                nc.vector.tensor_scalar(out=h1ln, in0=h1, scalar1=mv[:, 0:1],
                                        scalar2=rstd,
                                        op0=mybir.AluOpType.subtract,
                                        op1=mybir.AluOpType.mult)
            else:
                hn = p2.tile([D, H], F32, tag="hn")
                nc.vector.tensor_scalar(out=hn, in0=h1, scalar1=mv[:, 0:1],
                                        scalar2=rstd,
                                        op0=mybir.AluOpType.subtract,
                                        op1=mybir.AluOpType.mult)
                hg = p2.tile([D, H], F32, tag="hg")
                nc.vector.tensor_mul(out=hg, in0=hn, in1=fg_sb)
                nc.vector.tensor_add(out=h1ln, in0=hg, in1=fb_sb)

            pt = pst.tile([P, 4, NQ + 1], BF16, tag="tr")
            for j in range(4):
                nc.tensor.transpose(pt[:, j, 0:D], h1ln[:, j * P:(j + 1) * P],
                                    id16)
            h1T = p2.tile([P, 4, D], BF16, tag="h1T")
            nc.vector.tensor_copy(out=h1T, in_=pt[:, :, 0:D])

            ph2 = ffnp.tile([D, H], F32, tag="ph")
            for kc in range(4):
                nc.tensor.matmul(ph2, lhsT=h1T[:, kc, :], rhs=w2_sb[:, kc, :],
                                 start=(kc == 0),
                                 stop=(kc == 3 and "bf2" in skip))
            if "bf2" not in skip:
                nc.tensor.matmul(ph2, lhsT=ones_row[:, :D], rhs=bf2_sb,
                                 start=False, stop=True)
            h2 = p2.tile([D, H], F32, tag="h2")
            nc.scalar.activation(out=h2, in_=ph2,
                                 func=mybir.ActivationFunctionType.Gelu)

            prod = p2.tile([D, H], F32, tag="prod")
            nc.vector.tensor_mul(out=prod, in0=h2, in1=wo_sb)
            lsum = p2.tile([D, 1], F32, tag="lsum")
            nc.vector.reduce_sum(out=lsum, in_=prod, axis=mybir.AxisListType.X)
            nc.vector.tensor_scalar_add(out=logit_sb[:, b:b + 1], in0=lsum,
                                        scalar1=float(b_out_s))

        # ---- driver: pipelined projection stream with interleaved phase 2.
        # ctx matmuls trail the projection by one tile; each row's LN/
        # transpose block trails its last ctx matmul by one more tile so the
        # PE never waits on the DVE stats chain.
        PF = 4
        fused_rows = {}
        for t in range(TT):
            proj(t)
            if t + PF < TT:
                dma_tile(t + PF, nc.gpsimd)
            post(t)
            if t >= 1:
                ctx_mm(t - 1)
            if t >= 2 and (t - 2) % NT == NT - 1:
                b = (t - 2) // NT
                fused_rows[b] = row_ctx(b)
        ctx_mm(TT - 1)
        fused_rows[B_LOCAL - 1] = row_ctx(B_LOCAL - 1)
        for b in range(B_LOCAL):
            row_ffn(b, fused_rows.pop(b))

        nc.sync.dma_start(out=out.ap().rearrange("b d -> d b"), in_=logit_sb)


def _np(x):
    return np.asarray(x)


LAST_RESULT = None


def kernel(**inputs):
    from concourse.bass_utils import run_bass_kernel_spmd

    token_states = _np(inputs["token_states"]).astype(np.float32)
    mask = _np(inputs["attention_mask"])
    W_proj = _np(inputs["W_proj"]).astype(np.float32)
    b_proj = _np(inputs["b_proj"]).astype(np.float32)
    tln_g = _np(inputs["tln_g"]).astype(np.float32)
    tln_b = _np(inputs["tln_b"]).astype(np.float32)
    gln_g = _np(inputs["gln_g"]).astype(np.float32)
    gln_b = _np(inputs["gln_b"]).astype(np.float32)
    cln_g = _np(inputs["cln_g"]).astype(np.float32)
    cln_b = _np(inputs["cln_b"]).astype(np.float32)
    fln_g = _np(inputs["fln_g"]).astype(np.float32)
    fln_b = _np(inputs["fln_b"]).astype(np.float32)
    domain_queries = _np(inputs["domain_queries"]).astype(np.float32)
    global_query = _np(inputs["global_query"]).astype(np.float32)
    W_ff1 = _np(inputs["W_ff1"]).astype(np.float32)
    b_ff1 = _np(inputs["b_ff1"]).astype(np.float32)
    W_ff2 = _np(inputs["W_ff2"]).astype(np.float32)
    b_ff2 = _np(inputs["b_ff2"]).astype(np.float32)
    W_out = _np(inputs["W_out"]).astype(np.float32)
    b_out = _np(inputs["b_out"]).astype(np.float32)
    log_temperature = _np(inputs["log_temperature"]).astype(np.float32)

    Bq, Sq = mask.shape
    assert (Bq, Sq) == (B, S) and token_states.shape == (B, S, TOKD)

    # ---- host preprocessing ----
    compact = os.environ.get("KERNEL_COMPACT", "1") == "1"
    if compact:
        counts = mask.astype(bool).sum(axis=1)
        S_c = int(max(P, -(-int(counts.max()) // P) * P))
    else:
        S_c = S

    ts_c = np.zeros((B, S_c, TOKD), np.float32)
    padbias = np.full((B, S_c), -1e9, np.float32)
    if compact:
        for b in range(B):
            idx = np.flatnonzero(mask[b])
            n = len(idx)
            ts_c[b, :n] = token_states[b, idx]
            padbias[b, :n] = 0.0
    else:
        ts_c[:] = token_states
        padbias[:] = np.where(mask != 0, 0.0, -1e9)

    temp = float(np.clip(np.exp(log_temperature[0]), 0.3, 3.0))
    inv_t = 1.0 / temp
    wo_host = (W_out[:, 0] * inv_t).astype(np.float32)
    b_out_s = float(b_out[0] * inv_t)

    # queries folded into the projection: row 0 = global, 1..16 = domains
    q_all = np.concatenate([global_query[None, :], domain_queries], 0)  # [17,H]
    q_eff = q_all * tln_g[None, :]                                      # [17,H]
    sq = q_eff.sum(axis=1)                                              # [17]
    w_bar = W_proj.mean(axis=1)                                         # [TOKD]
    Q_hat = W_proj @ q_eff.T - w_bar[:, None] * sq[None, :]             # [TOKD,17]
    # pre-rstd per-query score offset from the projection bias. (The LN-bias
    # term tln_b.q is a post-rstd per-query constant — a uniform softmax
    # rescale per query — and cancels in the context LN, so it's dropped.)
    bq = (b_proj @ q_eff.T) - float(b_proj.mean()) * sq

    W_aug = np.concatenate([W_proj, Q_hat], axis=1)                 # [TOKD,529]
    w_host = W_aug.reshape(KC, P, WTOT).transpose(1, 0, 2)          # [128,KC,529]
    bprow_host = np.concatenate([b_proj, bq]).reshape(1, WTOT)

    cg_host = np.ones((NQ, H), np.float32)
    cb_host = np.zeros((NQ, H), np.float32)
    cg_host[0] = gln_g
    cb_host[0] = gln_b
    cg_host[1:] = cln_g
    cb_host[1:] = cln_b

    bf16 = ml_dtypes.bfloat16

    skip = set()
    if np.all(tln_g == 1) and np.all(tln_b == 0):
        skip.add("tln")
    if np.all(cg_host == 1) and np.all(cb_host == 0):
        skip.add("gcln")
    if np.all(fln_g == 1) and np.all(fln_b == 0):
        skip.add("fln")
    if np.all(b_proj == 0) and np.all(bq == 0):
        skip.add("bp")
    if np.all(b_ff1 == 0):
        skip.add("bf1")
    if np.all(b_ff2 == 0):
        skip.add("bf2")

    nc = build_nc(S_c, b_out_s, frozenset(skip))

    NT = S_c // P
    TT = B_LOCAL * NT

    shared = dict(
        wp=w_host.reshape(P, KC * WTOT).astype(bf16),
        bprow=bprow_host.astype(bf16),
        tg=tln_g[None, :], tb=tln_b[None, :],
        cg=cg_host, cb=cb_host,
        fg=fln_g[None, :], fb=fln_b[None, :],
        w1=W_ff1.reshape(8, P, H).transpose(1, 0, 2).reshape(P, 8 * H).astype(bf16),
        bf1=b_ff1[None, :].astype(bf16),
        w2=W_ff2.reshape(4, P, H).transpose(1, 0, 2).reshape(P, 4 * H).astype(bf16),
        bf2=b_ff2[None, :].astype(bf16),
        wo=wo_host[None, :],
    )

    in_maps = []
    for c in range(N_CORES):
        m = dict(shared)
        bs = slice(c * B_LOCAL, (c + 1) * B_LOCAL)
        tsc = ts_c[bs].reshape(TT, P, KC, P)          # [tile, s, c, p]
        m["ts"] = np.ascontiguousarray(
            tsc.transpose(0, 3, 2, 1)).reshape(TT * P, TOKD).astype(bf16)
        m["pb"] = padbias[bs].reshape(TT, P).astype(np.float32)
        in_maps.append(m)

    trace = os.environ.get("KERNEL_TRACE", "0") == "1"
    kw = {}
    if trace:
        kw = dict(trace=True, tmpdir=os.environ.get("KERNEL_TRACE_DIR") or None)
    res = run_bass_kernel_spmd(nc, in_maps, core_ids=list(range(N_CORES)), **kw)
    global LAST_RESULT
    LAST_RESULT = res
    outs = [res.results[c]["out"] for c in range(N_CORES)]
    return np.concatenate(outs, axis=0).astype(np.float32)


if __name__ == "__main__":
    pass
